# revision 1
# baseline (speedup 1.0000x reference)
"""Trainium2 Bass kernel for nn_DecomposingAttnProcessor.

Math (reference):
    q = hs @ Wq.T + bq;  k = ehs @ Wk.T + bk;  v = ehs @ Wv.T + bv
    scores = (q @ k.T) * dh**-0.5 per (bc, head)      [BC, H, S, T]
    w = softmax(scores over the COMPONENT axis)        (bc = c*B + b, C=4, B=2)
    w = w / (sum_t w + eps)
    out = (w @ v) -> [BC, S, D] -> @ Wo.T + bo

Distribution: shard S (4096 query tokens) across 8 cores, 512 each. Every
core handles all (bc, head) pairs for its S-slice, so the component softmax
group (same b, all c) stays on one core and each core emits complete output
rows (no cross-core reduction).

Layout strategy: everything stays transposed (host pre-transposes inputs):
    hsT [din, S], W*T [din, dout]  ->  qT [dout, S] (scaled by dh**-0.5)
    kT [dout, T], v [T, dv]        ->  scoresT [T, S] = kT_h.T-slice matmuls
    softmax elementwise on [T, S] tiles; row-normalization via ones-matmul
    rowsums [64, S] + reciprocal + multiply of the AV output outT [dh, S].
    attnT [f, S] feeds the O-projection directly as the moving operand:
    outT [dout, S]. The host un-transposes the final output.

All matmuls run in float32r (TF32, 1 cycle/row at N>=256 incl. K=64).
The w/v path of the AV + rowsum matmuls is bf16 (exp writes bf16 directly).
"""

import numpy as np
from contextlib import ExitStack

import concourse.bass as bass
import concourse.tile as tile
from concourse import bacc, mybir

F32 = mybir.dt.float32
F32R = mybir.dt.float32r
BF16 = mybir.dt.bfloat16

# problem shape (hardcoded per contract)
BC, S, D = 8, 4096, 1536
T = 154
C, B = 4, 2
H, DH = 24, 64
NCORES = 8
SL = S // NCORES          # 512 S-rows per core
NDI = D // 128            # 12 din chunks
NDO = D // 128            # 12 dout tiles
TP = 160                  # padded T stride in ehsT packing
T0, T1 = 128, T - 128     # T chunks: 128 + 26
SCALE = DH ** -0.5
import os
USE_GP_ADD = os.environ.get("DK_GP_ADD", "1") == "1"
USE_GP_MUL = os.environ.get("DK_GP_MUL", "1") == "1"
KV_N = BC * TP            # 1280
KV_BLOCKS = [(0, 512), (512, 512), (1024, 256)]   # N-blocks for kT proj
DV_BLOCKS = [(0, 512), (512, 512), (1024, 512)]   # N-blocks over D


def build_program():
    nc = bacc.Bacc("TRN2", target_bir_lowering=False, debug=False)

    # ---- external I/O (per core) ----
    hsT = nc.dram_tensor("hsT", [BC, D, SL], F32R, kind="ExternalInput").ap()
    ehsT = nc.dram_tensor("ehsT", [D, KV_N], F32R, kind="ExternalInput").ap()
    wqT = nc.dram_tensor("wqT", [D, D], F32R, kind="ExternalInput").ap()
    wkT = nc.dram_tensor("wkT", [D, D], F32R, kind="ExternalInput").ap()
    wvT = nc.dram_tensor("wvT", [D, D], F32R, kind="ExternalInput").ap()
    woT = nc.dram_tensor("woT", [D, D], F32R, kind="ExternalInput").ap()
    # biases laid out [128, 12] on host (bq pre-scaled by dh**-0.5)
    bqs = nc.dram_tensor("bqs", [128, NDO], F32, kind="ExternalInput").ap()
    bks = nc.dram_tensor("bks", [128, NDO], F32, kind="ExternalInput").ap()
    bos = nc.dram_tensor("bos", [128, NDO], F32, kind="ExternalInput").ap()
    bvr = nc.dram_tensor("bvr", [1, D], F32R, kind="ExternalInput").ap()
    onesr = nc.dram_tensor("onesr", [1, 128], F32R, kind="ExternalInput").ap()
    outT = nc.dram_tensor("outT", [BC, D, SL], F32, kind="ExternalOutput").ap()

    # ---- DRAM scratch (split per bc / per tile for fine-grained deps) ----
    kT_s = [nc.dram_tensor(f"kT_s{j}", [128, KV_N], F32R).ap()
            for j in range(NDO)]
    v_s = [nc.dram_tensor(f"v_s{bc}", [T, D], BF16).ap() for bc in range(BC)]
    qT_s = [nc.dram_tensor(f"qT_s{bc}", [NDO, 128, SL], F32R).ap()
            for bc in range(BC)]
    at_s = [nc.dram_tensor(f"at_s{bc}", [D, SL], F32R).ap()
            for bc in range(BC)]

    ENG_ADD = nc.gpsimd if USE_GP_ADD else nc.vector
    ENG_MUL = nc.gpsimd if USE_GP_MUL else nc.vector

    with tile.TileContext(nc) as tc, ExitStack() as ctx:
        const = ctx.enter_context(tc.tile_pool(name="const", bufs=1))
        ones_bf = const.tile([128, DH], BF16)
        nc.vector.memset(ones_bf[:], 1.0)
        ones_row = const.tile([1, 128], F32R)
        nc.sync.dma_start(ones_row[:], onesr[:])
        bq_t = const.tile([128, NDO], F32)
        bk_t = const.tile([128, NDO], F32)
        bo_t = const.tile([128, NDO], F32)
        bv_t = const.tile([1, D], F32R)
        nc.sync.dma_start(bq_t[:], bqs[:])
        nc.sync.dma_start(bk_t[:], bks[:])
        nc.sync.dma_start(bo_t[:], bos[:])
        nc.sync.dma_start(bv_t[:], bvr[:])

        # ================= P1: K and V projections =================
        with tc.tile_pool(name="eh", bufs=1) as ehp:
            eh = [ehp.tile([128, KV_N], F32R, tag=f"eh{_i}", name=f"eh{_i}") for _i in range(NDI)]
            for i in range(NDI):
                nc.sync.dma_start(eh[i][:], ehsT[i * 128:(i + 1) * 128, :])

            # ---- P1a: kT = Wk @ ehs^T (+bk), all bc packed along free ----
            with tc.tile_pool(name="p1a_w", bufs=1) as wp, \
                 tc.tile_pool(name="p1a_ps", bufs=4, space="PSUM") as pp, \
                 tc.tile_pool(name="p1a_o", bufs=3) as op:
                wk = [wp.tile([128, D], F32R, tag=f"wk{_i}", name=f"wk{_i}") for _i in range(NDI)]
                for i in range(NDI):
                    nc.sync.dma_start(wk[i][:], wkT[i * 128:(i + 1) * 128, :])
                for j in range(NDO):
                    for (nb0, nbl) in KV_BLOCKS:
                        ps = pp.tile([128, 512], F32, tag="ps")
                        for i in range(NDI):
                            nc.tensor.matmul(
                                ps[:, 0:nbl],
                                wk[i][:, j * 128:(j + 1) * 128],
                                eh[i][:, nb0:nb0 + nbl],
                                start=(i == 0), stop=(i == NDI - 1))
                        ot = op.tile([128, 512], F32R, tag="ot")
                        nc.scalar.activation(
                            ot[:, 0:nbl], ps[:, 0:nbl],
                            mybir.ActivationFunctionType.Identity,
                            bias=bk_t[:, j:j + 1])
                        nc.sync.dma_start(kT_s[j][:, nb0:nb0 + nbl],
                                          ot[:, 0:nbl])

            # ---- P1b: v = ehs @ Wv.T (+bv), stored bf16 ----
            with tc.tile_pool(name="p1b_w", bufs=1) as wp, \
                 tc.tile_pool(name="p1b_ps", bufs=4, space="PSUM") as pp, \
                 tc.tile_pool(name="p1b_o", bufs=3) as op:
                wv = [wp.tile([128, D], F32R, tag=f"wv{_i}", name=f"wv{_i}") for _i in range(NDI)]
                for i in range(NDI):
                    nc.sync.dma_start(wv[i][:], wvT[i * 128:(i + 1) * 128, :])
                for bc in (0, 2, 4, 6, 1, 3, 5, 7):
                    for (tt0, ttl) in [(0, T0), (T0, T1)]:
                        for (nb0, nbl) in DV_BLOCKS:
                            ps = pp.tile([128, 512], F32, tag="ps")
                            for i in range(NDI):
                                nc.tensor.matmul(
                                    ps[0:ttl, :],
                                    eh[i][:, bc * TP + tt0:bc * TP + tt0 + ttl],
                                    wv[i][:, nb0:nb0 + nbl],
                                    start=(i == 0), stop=False)
                            # bias row: K=1 matmul with ones-row lhsT
                            nc.tensor.matmul(
                                ps[0:ttl, :],
                                ones_row[0:1, 0:ttl],
                                bv_t[0:1, nb0:nb0 + nbl],
                                start=False, stop=True,
                                skip_group_check=True)
                            ot = op.tile([128, 512], BF16, tag="ot")
                            nc.scalar.copy(ot[0:ttl, :], ps[0:ttl, :])
                            nc.sync.dma_start(
                                v_s[bc][tt0:tt0 + ttl, nb0:nb0 + nbl],
                                ot[0:ttl, :])

        # ================= P2: Q projection (scaled) =================
        with tc.tile_pool(name="p2_w", bufs=1) as wp, \
             tc.tile_pool(name="p2_h", bufs=16) as hp, \
             tc.tile_pool(name="p2_ps", bufs=4, space="PSUM") as pp, \
             tc.tile_pool(name="p2_o", bufs=3) as op:
            wq = [wp.tile([128, D], F32R, tag=f"wq{_i}", name=f"wq{_i}") for _i in range(NDI)]
            for i in range(NDI):
                nc.sync.dma_start(wq[i][:], wqT[i * 128:(i + 1) * 128, :])
            for bc in (0, 2, 4, 6, 1, 3, 5, 7):
                ht = [hp.tile([128, SL], F32R, tag="ht", name=f"ht{_i}") for _i in range(NDI)]
                for i in range(NDI):
                    nc.sync.dma_start(ht[i][:], hsT[bc][i * 128:(i + 1) * 128, :])
                for j in range(NDO):
                    ps = pp.tile([128, SL], F32, tag="ps")
                    for i in range(NDI):
                        nc.tensor.matmul(ps[:], wq[i][:, j * 128:(j + 1) * 128],
                                         ht[i][:], start=(i == 0),
                                         stop=(i == NDI - 1))
                    qt = op.tile([128, SL], F32R, tag="qt")
                    nc.scalar.activation(
                        qt[:], ps[:], mybir.ActivationFunctionType.Identity,
                        bias=bq_t[:, j:j + 1], scale=SCALE)
                    nc.sync.dma_start(qT_s[bc][j], qt[:])

        # ================= P3: attention =================
        with tc.tile_pool(name="p3_in", bufs=18) as ip, \
             tc.tile_pool(name="p3_v", bufs=10) as vp, \
             tc.tile_pool(name="p3_e", bufs=2) as epool, \
             tc.tile_pool(name="p3_d", bufs=2) as dpool, \
             tc.tile_pool(name="p3_r", bufs=6) as rpool, \
             tc.tile_pool(name="p3_ps", bufs=2, space="PSUM") as pp, \
             tc.tile_pool(name="p3_ps2", bufs=1, space="PSUM") as pp2:
            for b in range(B):
                for hp_i in range(H // 2):          # head pair
                    # per-head loads keep every matmul operand at
                    # partition base 0 (non-zero PE tile positions are
                    # broken on this toolchain)
                    qt, kt, v0, v1 = [], [], [], []
                    for c in range(C):
                        bc = c * B + b
                        for j in range(2):
                            q = ip.tile([64, SL], F32R, tag="qt")
                            nc.sync.dma_start(
                                q[:], qT_s[bc][hp_i][j * 64:(j + 1) * 64, :])
                            qt.append(q)
                            k = ip.tile([64, T], F32R, tag="kt")
                            nc.sync.dma_start(
                                k[:], kT_s[hp_i][j * 64:(j + 1) * 64,
                                                 bc * TP:bc * TP + T])
                            kt.append(k)
                        va = vp.tile([128, 128], BF16, tag="v0")
                        nc.sync.dma_start(
                            va[:], v_s[bc][0:T0,
                                           hp_i * 128:(hp_i + 1) * 128])
                        v0.append(va)
                        vb = vp.tile([T1, 128], BF16, tag="v1")
                        nc.sync.dma_start(
                            vb[:], v_s[bc][T0:T,
                                           hp_i * 128:(hp_i + 1) * 128])
                        v1.append(vb)

                    # scores + exp for both heads of the pair
                    e0 = [epool.tile([128, 2 * SL], BF16, tag=f"e0_{_i}", name=f"e0_{_i}")
                          for _i in range(C)]
                    e1 = [epool.tile([T1, 2 * SL], BF16, tag=f"e1_{_i}", name=f"e1_{_i}")
                          for _i in range(C)]
                    for j in range(2):              # head within pair
                        for c in range(C):
                            qk, kk = qt[c * 2 + j], kt[c * 2 + j]
                            s0 = pp.tile([128, SL], F32, tag="s0")
                            nc.tensor.matmul(s0[:], kk[:, 0:T0], qk[:],
                                             start=True, stop=True)
                            nc.scalar.activation(
                                e0[c][:, j * SL:(j + 1) * SL], s0[:],
                                mybir.ActivationFunctionType.Exp)
                            s1 = pp.tile([T1, SL], F32, tag="s1")
                            nc.tensor.matmul(s1[:], kk[:, T0:T], qk[:],
                                             start=True, stop=True)
                            nc.scalar.activation(
                                e1[c][:, j * SL:(j + 1) * SL], s1[:],
                                mybir.ActivationFunctionType.Exp)

                    # component softmax: w_c = e_c / sum_c e_c  (in place;
                    # pure-bf16 adds/muls run on GpSimd to unload DVE)
                    for (ee, rows) in ((e0, 128), (e1, T1)):
                        t01 = dpool.tile([128, 2 * SL], BF16, tag="t01")
                        t23 = dpool.tile([128, 2 * SL], BF16, tag="t23")
                        dd = dpool.tile([128, 2 * SL], F32, tag="dd")
                        rbf = dpool.tile([128, 2 * SL], BF16, tag="rbf")
                        ENG_ADD.tensor_add(t01[0:rows, :], ee[0][:],
                                           ee[1][:])
                        ENG_ADD.tensor_add(t23[0:rows, :], ee[2][:],
                                           ee[3][:])
                        nc.vector.tensor_add(dd[0:rows, :], t01[0:rows, :],
                                             t23[0:rows, :])
                        nc.vector.reciprocal_approx_fast(dd[0:rows, :],
                                                         dd[0:rows, :])
                        nc.vector.tensor_copy(rbf[0:rows, :], dd[0:rows, :])
                        for c in range(C):
                            ENG_MUL.tensor_mul(ee[c][:], ee[c][:],
                                               rbf[0:rows, :])

                    # AV + rowsum + normalize per (c, head-pair packed wide)
                    for c in range(C):
                        bc = c * B + b
                        po = pp2.tile([64, 2 * SL], F32, tag="po")
                        pr = pp2.tile([64, 2 * SL], F32, tag="pr")
                        for j in range(2):
                            sl_ = slice(j * SL, (j + 1) * SL)
                            nc.tensor.matmul(
                                po[:, sl_], v0[c][:, j * 64:(j + 1) * 64],
                                e0[c][:, sl_], start=True, stop=False)
                            nc.tensor.matmul(
                                po[:, sl_], v1[c][:, j * 64:(j + 1) * 64],
                                e1[c][:, sl_], start=False, stop=True)
                            nc.tensor.matmul(
                                pr[:, sl_], ones_bf[0:128, 0:64],
                                e0[c][:, sl_], start=True, stop=False)
                            nc.tensor.matmul(
                                pr[:, sl_], ones_bf[0:T1, 0:64],
                                e1[c][:, sl_], start=False, stop=True)
                        rb = rpool.tile([64, 2 * SL], F32, tag="rb")
                        nc.vector.reciprocal_approx_fast(rb[:], pr[:])
                        at = rpool.tile([64, 2 * SL], F32R, tag="at")
                        nc.vector.tensor_mul(at[:], po[:], rb[:])
                        for j in range(2):
                            h = hp_i * 2 + j
                            nc.sync.dma_start(
                                at_s[bc][h * 64:(h + 1) * 64, :],
                                at[:, j * SL:(j + 1) * SL])

        # ================= P4: O projection =================
        with tc.tile_pool(name="p4_w", bufs=1) as wp, \
             tc.tile_pool(name="p4_a", bufs=16) as apool, \
             tc.tile_pool(name="p4_ps", bufs=4, space="PSUM") as pp, \
             tc.tile_pool(name="p4_o", bufs=3) as op:
            wo = [wp.tile([128, D], F32R, tag=f"wo{_i}", name=f"wo{_i}") for _i in range(NDI)]
            for i in range(NDI):
                nc.sync.dma_start(wo[i][:], woT[i * 128:(i + 1) * 128, :])
            for bc in (0, 2, 4, 6, 1, 3, 5, 7):
                att = [apool.tile([128, SL], F32R, tag="att", name=f"att{_i}")
                       for _i in range(NDI)]
                for i in range(NDI):
                    nc.sync.dma_start(att[i][:],
                                      at_s[bc][i * 128:(i + 1) * 128, :])
                for j in range(NDO):
                    ps = pp.tile([128, SL], F32, tag="ps")
                    for i in range(NDI):
                        nc.tensor.matmul(ps[:], wo[i][:, j * 128:(j + 1) * 128],
                                         att[i][:], start=(i == 0),
                                         stop=(i == NDI - 1))
                    ot = op.tile([128, SL], F32, tag="ot")
                    nc.scalar.activation(
                        ot[:], ps[:], mybir.ActivationFunctionType.Identity,
                        bias=bo_t[:, j:j + 1])
                    nc.sync.dma_start(outT[bc][j * 128:(j + 1) * 128, :],
                                      ot[:])

    nc.compile()
    return nc


_NC_CACHE = None


def _get_program():
    global _NC_CACHE
    if _NC_CACHE is None:
        _NC_CACHE = build_program()
    return _NC_CACHE


def make_in_maps(hidden_states, encoder_hidden_states, Wq, bq, Wk, bk,
                 Wv, bv, Wo, bo):
    """Host-side shard + transpose prep. Returns per-core input dicts."""
    hs = np.ascontiguousarray(hidden_states, dtype=np.float32)
    ehs = np.ascontiguousarray(encoder_hidden_states, dtype=np.float32)

    ehsT = np.zeros((D, KV_N), dtype=np.float32)
    for bc in range(BC):
        ehsT[:, bc * TP:bc * TP + T] = ehs[bc].T

    shared = {
        "ehsT": ehsT,
        "wqT": np.ascontiguousarray(Wq.T, dtype=np.float32),
        "wkT": np.ascontiguousarray(Wk.T, dtype=np.float32),
        "wvT": np.ascontiguousarray(Wv.T, dtype=np.float32),
        "woT": np.ascontiguousarray(Wo.T, dtype=np.float32),
        "bqs": np.ascontiguousarray(
            (np.asarray(bq, np.float32) * SCALE).reshape(NDO, 128).T),
        "bks": np.ascontiguousarray(
            np.asarray(bk, np.float32).reshape(NDO, 128).T),
        "bos": np.ascontiguousarray(
            np.asarray(bo, np.float32).reshape(NDO, 128).T),
        "bvr": np.asarray(bv, np.float32).reshape(1, D),
        "onesr": np.ones((1, 128), np.float32),
    }
    in_maps = []
    for core in range(NCORES):
        sl = slice(core * SL, (core + 1) * SL)
        hsT = np.ascontiguousarray(hs[:, sl, :].transpose(0, 2, 1))
        in_maps.append({**shared, "hsT": hsT})
    return in_maps


def run_sharded(inputs, trace=False, tmpdir=None, trace_cores=None):
    from concourse.bass_utils import run_bass_kernel_spmd
    nc = _get_program()
    in_maps = make_in_maps(**inputs)
    res = run_bass_kernel_spmd(nc, in_maps, list(range(NCORES)), trace=trace,
                               tmpdir=tmpdir, trace_cores=trace_cores)
    out = np.empty((BC, S, D), dtype=np.float32)
    for core in range(NCORES):
        sl = slice(core * SL, (core + 1) * SL)
        out[:, sl, :] = res.results[core]["outT"].transpose(0, 2, 1)
    return out, res


def kernel(**inputs):
    out, _ = run_sharded(inputs, trace=False)
    return out

